# revision 43
# baseline (speedup 1.0000x reference)
"""3-layer GAT on Trainium2, 8-core SPMD Bass kernel.

Strategy (dst-partitioned, edge-gather based):
  - Nodes partitioned contiguously across 8 cores (6250/core). Each core owns
    all edges whose dst lands in its range, so segment-softmax and the
    weighted scatter-sum are core-local.
  - Per layer: each core computes z_aug = x @ [W.T | W.T a_src | W.T a_dst]
    for its node rows, writes fp16 table rows [z | 1 | 0 | s(f32) | t(f32)],
    then an AllGather replicates the full node table to every core.
  - Edge phase: edges are grouped by 64-dst blocks (host-side, index-only
    preprocessing). For each supertile (8 blocks) one dma_gather per
    src-half pulls the z_aug rows of all edge sources. Attention:
    e = s[src] + t[dst], leaky_relu, exp (no max-shift needed: |e| is small,
    f32 exp is exact softmax math since softmax is shift-invariant).
    Per-dst sums via PE matmul with a one-hot*ex matrix: U = PT_ex.T @ Zg,
    with a constant-1 table column producing the denominator for free.
  - out rows = U[:, :D] / max(denom, 1e-9); activation; transpose to feed the
    next layer's matmul.

Host preprocessing touches only src/dst (graph format conversion: grouping,
padding, int16 index packing) and weight layout; all float graph compute
happens on device.
"""

import math
import sys

import numpy as np

sys.path.insert(0, "/opt/trn_rl_repo")

import ml_dtypes  # noqa: E402

# --- problem constants (hardcoded per contest rules) ---
N_NODES = 50000
N_EDGES = 800000
DIM_IN = 256
DIM_HID = 256
DIM_OUT = 128
N_CORES = 8

BLOCK = 64  # dst nodes per mask block (seg ids 0..63)
STB = 4     # blocks per supertile
MAXCK = 4  # max chunks (x128 descriptors) per dma_gather call (ring capacity)
P = 128

NEG_SLOPE = 0.01


def _cdiv(a, b):
    return -(-a // b)


# ---------------------------------------------------------------------------
# host-side graph plan (pure index preprocessing)
# ---------------------------------------------------------------------------

def build_plan(src, dst, n_nodes, n_cores):
    src = np.asarray(src).astype(np.int64)
    dst = np.asarray(dst).astype(np.int64)
    assert n_nodes % n_cores == 0
    npc = n_nodes // n_cores
    npc2 = npc // 2
    assert n_cores * npc2 <= 32767

    n_blocks = _cdiv(npc, BLOCK)
    n_st = _cdiv(n_blocks, STB)
    assert n_blocks % 2 == 0 or BLOCK * (n_blocks - 1) < npc  # pairs exist
    c = dst // npc
    loc = dst - c * npc
    b = loc // BLOCK
    sg = loc % BLOCK
    r_src = src % npc
    hf = (r_src >= npc2).astype(np.int64)
    gidx = (src // npc) * npc2 + (r_src % npc2)

    key = (c * n_blocks + b) * 2 + hf
    cnt = np.bincount(key, minlength=n_cores * n_blocks * 2).reshape(
        n_cores, n_blocks, 2
    )
    mx = cnt.max(axis=0)  # [n_blocks, 2]
    ck = (mx + P - 1) // P  # chunks per (block, half), shared across cores
    dead = ck.sum(axis=1) == 0
    ck[dead, 0] = 1  # keep >=1 chunk per block so PSUM init happens

    order = np.lexsort((src, hf, b, c))
    gsize = cnt.reshape(-1)
    gstart = np.zeros_like(gsize)
    gstart[1:] = np.cumsum(gsize)[:-1]

    sts = []
    seg_cols = lo_cols = hi_cols = 0
    for st in range(n_st):
        bs = list(range(st * STB, min((st + 1) * STB, n_blocks)))
        clo = [int(ck[bb, 0]) for bb in bs]
        chi = [int(ck[bb, 1]) for bb in bs]
        CLO, CHI = sum(clo), sum(chi)
        blk = []
        for bi in range(len(bs)):
            blk += [bi] * clo[bi]
        for bi in range(len(bs)):
            blk += [bi] * chi[bi]
        sts.append(
            dict(
                bs=bs,
                clo=clo,
                chi=chi,
                CLO=CLO,
                CHI=CHI,
                CTOT=CLO + CHI,
                seg_off=seg_cols,
                lo_off=lo_cols,
                hi_off=hi_cols,
                blk=blk,
            )
        )
        seg_cols += CLO + CHI
        lo_cols += CLO * 8
        hi_cols += CHI * 8

    def wrap16(a):
        # idx j -> partition j%16, col j//16; replicated to 8 groups of 16
        S = len(a) // 16
        w = a.reshape(S, 16).T
        return np.tile(w, (8, 1))

    idxlo = np.zeros((n_cores, P, lo_cols), np.int16)
    idxhi = np.zeros((n_cores, P, hi_cols), np.int16)
    seg = np.full((n_cores, P, seg_cols), 255.0, np.float32)
    # transposed one-hot per chunk: rows = (block%2)*64 + dst-slot, cols = edges
    pt0t = np.zeros((n_cores, P, seg_cols * P), ml_dtypes.float8_e4m3)

    for cc_ in range(n_cores):
        for std in sts:
            lo_list = np.zeros(std["CLO"] * P, np.int64)
            hi_list = np.zeros(std["CHI"] * P, np.int64)
            seg_list = np.full(std["CTOT"] * P, 255, np.int64)
            blk_of_chunk = np.zeros(std["CTOT"], np.int64)
            lo_pos = 0
            slot = 0
            for bi, bb in enumerate(std["bs"]):
                n = cnt[cc_, bb, 0]
                s0 = gstart[(cc_ * n_blocks + bb) * 2 + 0]
                e = order[s0 : s0 + n]
                lo_list[lo_pos : lo_pos + n] = gidx[e]
                seg_list[slot : slot + n] = sg[e]
                blk_of_chunk[slot // P : slot // P + std["clo"][bi]] = bi
                lo_pos += std["clo"][bi] * P
                slot += std["clo"][bi] * P
            hi_pos = 0
            for bi, bb in enumerate(std["bs"]):
                n = cnt[cc_, bb, 1]
                s0 = gstart[(cc_ * n_blocks + bb) * 2 + 1]
                e = order[s0 : s0 + n]
                hi_list[hi_pos : hi_pos + n] = gidx[e]
                seg_list[slot : slot + n] = sg[e]
                blk_of_chunk[slot // P : slot // P + std["chi"][bi]] = bi
                hi_pos += std["chi"][bi] * P
                slot += std["chi"][bi] * P
            if std["CLO"]:
                idxlo[cc_, :, std["lo_off"] : std["lo_off"] + std["CLO"] * 8] = wrap16(
                    lo_list
                )
            if std["CHI"]:
                idxhi[cc_, :, std["hi_off"] : std["hi_off"] + std["CHI"] * 8] = wrap16(
                    hi_list
                )
            seg[cc_, :, std["seg_off"] : std["seg_off"] + std["CTOT"]] = (
                seg_list.reshape(std["CTOT"], P).T
            )
            # pt0t: per chunk c, one-hot column per edge at row (bi%2)*64+sg
            sl = seg_list.reshape(std["CTOT"], P)
            rows = (blk_of_chunk[:, None] % 2) * BLOCK + sl  # [CTOT, P]
            valid = sl < BLOCK
            cidx, eidx = np.nonzero(valid)
            pt = np.zeros((std["CTOT"], P, P), np.float16)  # [chunk, row, edge]
            pt[cidx, rows[cidx, eidx], eidx] = 1.0
            pt0t[cc_, :, std["seg_off"] * P : (std["seg_off"] + std["CTOT"]) * P] = (
                pt.transpose(1, 0, 2).reshape(P, std["CTOT"] * P)
            )

    meta = dict(
        n_cores=n_cores,
        n_nodes=n_nodes,
        npc=npc,
        npc2=npc2,
        n_blocks=n_blocks,
        n_rb=_cdiv(npc, P),
        sts=sts,
        lo_cols=lo_cols,
        hi_cols=hi_cols,
        seg_cols=seg_cols,
        ctot_max=max(s["CTOT"] for s in sts),
        ck_max=int(ck.max()),
    )
    per_core = dict(
        idxlo=idxlo,
        idxhi=idxhi,
        seg=seg.astype(ml_dtypes.bfloat16),
        pt0t=pt0t,
    )
    return meta, per_core


def const_inputs():
    iota = np.tile(np.arange(BLOCK, dtype=np.float32), (P, 1)).astype(
        ml_dtypes.bfloat16
    )
    id16 = np.eye(P, dtype=np.float16)
    return {"iota": iota, "id16": id16}


def build_waug(W, A):
    d_out = W.shape[0]
    Wt = W.T.astype(np.float64)
    a_s = A[0, :d_out].astype(np.float64)
    a_d = A[0, d_out:].astype(np.float64)
    waug = np.concatenate([Wt, (Wt @ a_s)[:, None], (Wt @ a_d)[:, None]], axis=1)
    return waug.astype(np.float16)


# ---------------------------------------------------------------------------
# device program
# ---------------------------------------------------------------------------

def build_nc(meta, dims, debug=False, timing_single_core=False, no_collective=False, no_gather=False, no_mask=False, cap_g=None, cap_m=None):
    import concourse.bacc as bacc
    import concourse.bass as bass
    import concourse.mybir as mybir
    import concourse.tile as tile
    from concourse.library_config import mlp

    dt = mybir.dt
    AP = bass.AP
    d_in, d_hid, d_out = dims
    npc = meta["npc"]
    npc2 = meta["npc2"]
    N = meta["n_nodes"]
    n_rb = meta["n_rb"]
    n_cores = meta["n_cores"]
    KCH = d_in // P  # contraction chunks (2)
    assert d_in == d_hid == 2 * P and d_out == P

    #        (Din,  Dout,  act,    stride, elem)
    LYR = [
        (d_in, d_hid, "tanh", 384, 384),
        (d_hid, d_hid, "elu", 384, 384),
        (d_hid, d_out, None, 256, 256),
    ]

    nc = bacc.Bacc(
        "TRN2", target_bir_lowering=False, debug=debug,
        num_devices=1 if timing_single_core else n_cores,
        num_swdge_queues=4,
    )

    h_in = nc.dram_tensor("h", [npc, d_in], dt.float32, kind="ExternalInput")
    w_in = [
        nc.dram_tensor(f"w{l}", [LYR[l][0], LYR[l][1] + 2], dt.float16,
                       kind="ExternalInput")
        for l in range(3)
    ]
    ixlo_in = nc.dram_tensor("idxlo", [P, meta["lo_cols"]], dt.int16,
                             kind="ExternalInput")
    ixhi_in = nc.dram_tensor("idxhi", [P, meta["hi_cols"]], dt.int16,
                             kind="ExternalInput")
    seg_in = nc.dram_tensor("seg", [P, meta["seg_cols"]], dt.bfloat16,
                            kind="ExternalInput")
    pt0t_in = nc.dram_tensor("pt0t", [P, meta["seg_cols"] * P], dt.float8e4,
                             kind="ExternalInput")
    iota_in = nc.dram_tensor("iota", [P, BLOCK], dt.bfloat16, kind="ExternalInput")
    id16_in = nc.dram_tensor("id16", [P, P], dt.float16, kind="ExternalInput")
    out_t = nc.dram_tensor("out", [npc, d_out], dt.float32, kind="ExternalOutput")

    agi = [nc.dram_tensor(f"agi{l}", [npc, LYR[l][3]], dt.float16) for l in range(3)]
    agoA = [
        nc.dram_tensor(f"agoA{l}", [npc2 * n_cores, LYR[l][3]], dt.float16,
                       addr_space="Shared")
        for l in range(3)
    ]
    agoB = [
        nc.dram_tensor(f"agoB{l}", [npc2 * n_cores, LYR[l][3]], dt.float16,
                       addr_space="Shared")
        for l in range(3)
    ]

    def bc_mid(ap2, n):
        # [P, W] -> [P, n, W] broadcasting a middle dim
        return AP(ap2.tensor, ap2.offset, [ap2.ap[0], [0, n], ap2.ap[1]])

    def bc_last(ap2, n):
        # [P, W] -> [P, W, n] broadcasting the last dim
        return AP(ap2.tensor, ap2.offset, [ap2.ap[0], ap2.ap[1], [0, n]])

    with tile.TileContext(nc) as tc:
        import contextlib

        ctx = contextlib.ExitStack()
        with ctx:
            pers = ctx.enter_context(tc.tile_pool(name="pers", bufs=1))
            pg = ctx.enter_context(tc.tile_pool(name="pg", bufs=4))
            ppt = ctx.enter_context(tc.tile_pool(name="ppt", bufs=3))
            pptt = ctx.enter_context(tc.tile_pool(name="pptt", bufs=3))
            psm = ctx.enter_context(tc.tile_pool(name="psm", bufs=6))
            pz = ctx.enter_context(tc.tile_pool(name="pz", bufs=3))
            px = ctx.enter_context(tc.tile_pool(name="px", bufs=2))
            psum_z = ctx.enter_context(tc.tile_pool(name="psz", bufs=1, space="PSUM"))
            psum_u = ctx.enter_context(tc.tile_pool(name="psu", bufs=3, space="PSUM"))
            psum_ts = ctx.enter_context(tc.tile_pool(name="psts", bufs=2, space="PSUM"))
            psum_tr = ctx.enter_context(tc.tile_pool(name="pstr", bufs=2, space="PSUM"))

            nc.gpsimd.load_library(mlp)

            # persistent state
            seg_sb = pers.tile([P, meta["seg_cols"]], dt.bfloat16, tag="seg", name="seg_sb")
            ixlo_sb = pers.tile([P, meta["lo_cols"]], dt.int16, tag="ixlo", name="ixlo_sb")
            ixhi_sb = pers.tile([P, meta["hi_cols"]], dt.int16, tag="ixhi", name="ixhi_sb")
            W_sb = [pers.tile([P, KCH, LYR[l][1] + 2], dt.float16, tag=f"w{l}", name=f"wsb{l}")
                    for l in range(3)]
            iota_sb = pers.tile([P, BLOCK], dt.bfloat16, tag="iota", name="iota_sb")
            id16 = pers.tile([P, P], dt.float16, tag="id16", name="id16")
            tcols2 = [
                pers.tile([P, n_rb], dt.float16, tag=f"tcols{i}", name=f"tcols{i}")
                for i in range(2)
            ]
            npad = n_rb * P
            xT = [
                [pers.tile([P, npad], dt.float16, tag=f"xT{par}_{k}", name=f"xT{par}_{k}") for k in range(KCH)]
                for par in range(2)
            ]

            nc.sync.dma_start(out=seg_sb[:], in_=seg_in[:, :])
            nc.sync.dma_start(out=ixlo_sb[:], in_=ixlo_in[:, :])
            nc.sync.dma_start(out=ixhi_sb[:], in_=ixhi_in[:, :])
            for l in range(3):
                nc.sync.dma_start(
                    out=W_sb[l][:],
                    in_=w_in[l].ap().rearrange("(k p) d -> p k d", p=P),
                )
            nc.sync.dma_start(out=iota_sb[:], in_=iota_in[:, :])
            nc.sync.dma_start(out=id16[:], in_=id16_in[:, :])
            nc.vector.memset(tcols2[0][:], 0.0)
            nc.vector.memset(tcols2[1][:], 0.0)

            # ---- layer-1 input: load h, cast fp16, transpose to xT[0] ----
            for rb in range(n_rb):
                rows = min(P, npc - rb * P)
                ht = pz.tile([P, d_in], dt.float32, tag="ht", name="ht")
                nc.sync.dma_start(out=ht[:rows], in_=h_in[rb * P : rb * P + rows, :])
                h16 = pz.tile([P, d_in], dt.float16, tag="h16", name="h16")
                nc.vector.tensor_copy(out=h16[:rows], in_=ht[:rows])
                for k in range(KCH):
                    ps = psum_tr.tile([P, P], dt.float16, tag="tps", name="tps")
                    nc.tensor.transpose(
                        out=ps[:P, :rows],
                        in_=h16[:rows, k * P : (k + 1) * P],
                        identity=id16[:rows, :rows],
                    )
                    nc.vector.tensor_copy(
                        out=xT[0][k][:, rb * P : rb * P + rows], in_=ps[:, :rows]
                    )

            # ---- z-phase block (one 128-row slab of layer l) ----
            def z_block(l, rb):
                Din, Dout, _, _, _ = LYR[l]
                DU = Dout + 2
                ASM = Dout + 6
                xin = xT[l % 2]
                rows = min(P, npc - rb * P)
                zp = psum_z.tile([P, 258], dt.float32, tag="zp", name="zp")
                for k in range(KCH):
                    nc.tensor.matmul(
                        out=zp[:rows, :DU],
                        lhsT=xin[k][:, rb * P : rb * P + rows],
                        rhs=W_sb[l][:, k, :DU],
                        start=(k == 0),
                        stop=(k == KCH - 1),
                    )
                asm = pz.tile([P, 262], dt.float16, tag="asm", name="asm")
                nc.scalar.copy(out=asm[:rows, 0:Dout], in_=zp[:rows, 0:Dout])
                nc.vector.memset(asm[:rows, Dout : Dout + 1], 1.0)
                nc.vector.memset(asm[:rows, Dout + 1 : Dout + 2], 0.0)
                nc.scalar.copy(
                    out=asm[:rows, Dout + 2 : Dout + 6].bitcast(dt.float32),
                    in_=zp[:rows, Dout : Dout + 2],
                )
                nc.scalar.copy(
                    out=tcols2[l % 2][:rows, rb : rb + 1],
                    in_=zp[:rows, Dout + 1 : Dout + 2],
                )
                nc.sync.dma_start(
                    out=agi[l][rb * P : rb * P + rows, 0:ASM], in_=asm[:rows, 0:ASM]
                )

            rb_a = _cdiv(npc2, P) - 1

            def emit_ag(ll, part):
                src_ap = agi[ll].ap()[part * npc2 : (part + 1) * npc2, :]
                dst = (agoA if part == 0 else agoB)[ll]
                if timing_single_core or no_collective:
                    nc.sync.dma_start(out=dst.ap()[0:npc2, :], in_=src_ap)
                else:
                    nc.gpsimd.collective_compute(
                        "AllGather",
                        bass.mybir.AluOpType.bypass,
                        replica_groups=[list(range(n_cores))],
                        ins=[src_ap.opt()],
                        outs=[dst.ap().opt()],
                    )

            for rb in range(n_rb):
                z_block(0, rb)
                if rb == rb_a:
                    emit_ag(0, 0)
            emit_ag(0, 1)

            # ---- layers ----
            for l in range(3):
                Din, Dout, act, STRIDE, ELEM = LYR[l]
                DU = Dout + 2
                ASM = Dout + 6
                SOFF = Dout + 2
                tcols = tcols2[l % 2]
                xout = xT[(l + 1) % 2]
                last = l == 2

                lo_tab = agoA[l].ap()[:, 0:ELEM]
                hi_tab = agoB[l].ap()[:, 0:ELEM]
                qrr = 0
                n_st_l = len(meta["sts"])
                S = {}

                # edge phase, software-pipelined: iteration `it` issues
                # ptt+lo-gathers for supertile `it`, then hi-gathers and all
                # compute for supertile `it-1` (hi lags so the lo-gathers can
                # start as soon as the A-half AllGather lands).
                for it in range(n_st_l + 1):
                    if it < n_st_l:
                        std = meta["sts"][it]
                        CLO, CHI, CTOT = std["CLO"], std["CHI"], std["CTOT"]
                        g = pg.tile([P, CTOT, ELEM], dt.float16, tag="g", name="g")
                        ptt = pptt.tile([P, CTOT, P], dt.float8e4, tag="ptt", name="ptt")
                        nc.sync.dma_start(
                            out=ptt[:],
                            in_=pt0t_in.ap()[
                                :, std["seg_off"] * P : (std["seg_off"] + CTOT) * P
                            ].rearrange("p (c e) -> p c e", e=P),
                        )
                        for c0 in range(0, (min(cap_g, CLO) if cap_g else CLO), MAXCK):
                            nck = min(MAXCK, CLO - c0)
                            off = std["lo_off"] + c0 * 8
                            nc.gpsimd.dma_gather(
                                g[:, c0 : c0 + nck, :],
                                lo_tab,
                                ixlo_sb[:, off : off + nck * 8],
                                nck * P,
                                nck * P,
                                ELEM,
                                elem_step=STRIDE,
                                queue_num=qrr % 4,
                                single_packet=False,
                            )
                            qrr += 1
                        S[it] = (g, ptt)
                    if it == 0:
                        continue
                    sti = it - 1
                    std = meta["sts"][sti]
                    CLO, CHI, CTOT = std["CLO"], std["CHI"], std["CTOT"]
                    g, ptt = S.pop(sti)
                    for c0 in range(0, (min(cap_g, CHI) if cap_g else CHI), MAXCK):
                        nck = min(MAXCK, CHI - c0)
                        off = std["hi_off"] + c0 * 8
                        nc.gpsimd.dma_gather(
                            g[:, CLO + c0 : CLO + c0 + nck, :],
                            hi_tab,
                            ixhi_sb[:, off : off + nck * 8],
                            nck * P,
                            nck * P,
                            ELEM,
                            elem_step=STRIDE,
                            queue_num=qrr % 4,
                            single_packet=False,
                        )
                        qrr += 1
                    # chunk offsets (local to st) per block
                    lopos = np.concatenate([[0], np.cumsum(std["clo"])]).astype(int)
                    hipos = CLO + np.concatenate([[0], np.cumsum(std["chi"])]).astype(int)
                    bs = std["bs"]

                    # supertile-wide attention scalars:
                    # t per edge via PE matvec on transposed one-hot, then
                    # e = s + t, exp(lrelu(e)) = max(exp(e), exp(0.01 e))
                    tselp = psum_ts.tile([P, CTOT], dt.float32, tag="tselp", name="tselp")
                    for c in range(CTOT):
                        rbc = 2 * sti + std["blk"][c] // 2
                        nc.tensor.matmul(
                            out=tselp[:, c : c + 1],
                            lhsT=ptt[:, c, :],
                            rhs=tcols[:, rbc : rbc + 1],
                            start=True,
                            stop=True,
                        )
                    sv = g[:, 0:CTOT, SOFF : SOFF + 2].bitcast(dt.float32)
                    sv2 = AP(sv.tensor, sv.offset, [sv.ap[0], sv.ap[1]])
                    e0 = psm.tile([P, CTOT], dt.float32, tag="e0", name="e0")
                    nc.vector.tensor_tensor(
                        out=e0[:], in0=sv2, in1=tselp[:],
                        op=bass.mybir.AluOpType.add,
                    )
                    exa = psm.tile([P, CTOT], dt.float32, tag="exa", name="exa")
                    nc.scalar.activation(
                        exa[:], e0[:], bass.mybir.ActivationFunctionType.Exp
                    )
                    exc = psm.tile([P, CTOT], dt.float32, tag="exc", name="exc")
                    nc.scalar.activation(
                        exc[:], e0[:],
                        bass.mybir.ActivationFunctionType.Exp,
                        scale=NEG_SLOPE,
                    )
                    exb = psm.tile([P, CTOT], dt.bfloat16, tag="exb", name="exb")
                    nc.vector.tensor_tensor(
                        out=exb[:], in0=exa[:], in1=exc[:],
                        op=bass.mybir.AluOpType.max,
                    )
                    for pi in range(0, len(bs), 2):
                        pair = bs[pi : pi + 2]
                        node0 = pair[0] * BLOCK
                        rbi = node0 // P
                        prows = min(2 * BLOCK, npc - node0)
                        U = psum_u.tile([P, DU], dt.float32, tag="U", name="U")
                        if not last:
                            xp = px.tile([P, 256], dt.float32, tag="xp", name="xp")
                        for si, bb in enumerate(pair):
                            bi = pi + si
                            bn = min(BLOCK, npc - bb * BLOCK)
                            s0 = si * BLOCK
                            segs = []
                            if std["clo"][bi]:
                                segs.append((int(lopos[bi]), std["clo"][bi]))
                            if std["chi"][bi]:
                                segs.append((int(hipos[bi]), std["chi"][bi]))
                            total_ck = sum(cc for _, cc in segs)
                            done = 0
                            for cs, cc in ([(cs0, min(cap_m, cc0)) for cs0, cc0 in segs] if cap_m else segs):
                                seg_v = seg_sb[:, std["seg_off"] + cs : std["seg_off"] + cs + cc]
                                pt0 = ppt.tile([P, cc, BLOCK], dt.bfloat16, tag="pt0", name="pt0")
                                nc.vector.tensor_tensor(
                                    out=pt0[:],
                                    in0=bc_last(seg_v, BLOCK),
                                    in1=bc_mid(iota_sb[:], cc),
                                    op=bass.mybir.AluOpType.is_equal,
                                )
                                ptx = ppt.tile([P, cc, BLOCK], dt.bfloat16, tag="ptx", name="ptx")
                                nc.vector.tensor_tensor(
                                    out=ptx[:],
                                    in0=pt0[:],
                                    in1=bc_last(exb[:, cs : cs + cc], BLOCK),
                                    op=bass.mybir.AluOpType.mult,
                                )
                                for k in range(cc):
                                    nc.tensor.matmul(
                                        out=U[s0 : s0 + BLOCK, :DU],
                                        lhsT=ptx[:, k, :],
                                        rhs=g[:, cs + k, 0:DU],
                                        start=(done == 0),
                                        stop=(done == total_ck - 1),
                                        tile_position=(0, s0),
                                    )
                                    done += 1
                        # normalize whole pair at once
                        den = psm.tile([P, 1], dt.float32, tag="den", name="den")
                        nc.vector.tensor_scalar(
                            out=den[0:prows],
                            in0=U[0:prows, Dout : Dout + 1],
                            scalar1=1e-9,
                            scalar2=None,
                            op0=bass.mybir.AluOpType.max,
                        )
                        rec = psm.tile([P, 1], dt.float32, tag="rec", name="rec")
                        nc.vector.reciprocal(rec[0:prows], den[0:prows])
                        if last:
                            ox = psm.tile([P, d_out], dt.float32, tag="ox", name="ox")
                            nc.vector.tensor_scalar(
                                out=ox[0:prows],
                                in0=U[0:prows, 0:d_out],
                                scalar1=rec[0:prows],
                                scalar2=None,
                                op0=bass.mybir.AluOpType.mult,
                            )
                            nc.sync.dma_start(
                                out=out_t[node0 : node0 + prows, :],
                                in_=ox[0:prows, :],
                            )
                            continue
                        nc.vector.tensor_scalar(
                            out=xp[0:prows, :Dout],
                            in0=U[0:prows, 0:Dout],
                            scalar1=rec[0:prows],
                            scalar2=None,
                            op0=bass.mybir.AluOpType.mult,
                        )
                        # activation + transpose into xout
                        a16 = px.tile([P, 256], dt.float16, tag="a16", name="a16")
                        if act == "tanh":
                            nc.scalar.activation(
                                a16[:prows], xp[:prows],
                                bass.mybir.ActivationFunctionType.Tanh,
                            )
                        else:  # elu = relu(x) + exp(-relu(-x)) - 1
                            mn = px.tile([P, 256], dt.float32, tag="mn", name="mn")
                            nc.scalar.activation(
                                mn[:prows], xp[:prows],
                                bass.mybir.ActivationFunctionType.Relu,
                                scale=-1.0,
                            )
                            ee = px.tile([P, 256], dt.float32, tag="ee", name="ee")
                            nc.scalar.activation(
                                ee[:prows], mn[:prows],
                                bass.mybir.ActivationFunctionType.Exp,
                                scale=-1.0,
                            )
                            mx2 = px.tile([P, 256], dt.float32, tag="mx2", name="mx2")
                            nc.scalar.activation(
                                mx2[:prows], xp[:prows],
                                bass.mybir.ActivationFunctionType.Relu,
                            )
                            nc.vector.tensor_tensor(
                                out=ee[:prows], in0=ee[:prows], in1=mx2[:prows],
                                op=bass.mybir.AluOpType.add,
                            )
                            nc.vector.tensor_scalar(
                                out=a16[:prows], in0=ee[:prows], scalar1=-1.0,
                                scalar2=None, op0=bass.mybir.AluOpType.add,
                            )
                        for k in range(KCH):
                            ps = psum_tr.tile([P, P], dt.float16, tag="tps", name="tps")
                            nc.tensor.transpose(
                                out=ps[:P, :prows],
                                in_=a16[:prows, k * P : (k + 1) * P],
                                identity=id16[:prows, :prows],
                            )
                            nc.scalar.copy(
                                out=xout[k][:, node0 : node0 + prows],
                                in_=ps[:, :prows],
                            )
                    if not last:
                        # overlap next layer's z-phase with this edge phase:
                        # pair rb of supertile sti is exactly row-block rb
                        for rb in range(2 * sti, min(2 * sti + 2, n_rb)):
                            z_block(l + 1, rb)
                            if rb == rb_a:
                                emit_ag(l + 1, 0)
                if not last:
                    emit_ag(l + 1, 1)

    nc.compile()
    return nc


# ---------------------------------------------------------------------------
# entry point
# ---------------------------------------------------------------------------

_CACHE = {}


def _prepare(src, dst, n_nodes):
    key = (int(n_nodes), src.tobytes(), dst.tobytes())
    kh = hash(key)
    if kh not in _CACHE:
        meta, per_core = build_plan(src, dst, n_nodes, N_CORES)
        nc = build_nc(meta, (DIM_IN, DIM_HID, DIM_OUT))
        _CACHE[kh] = (meta, per_core, nc)
    return _CACHE[kh]


def kernel(h, src, dst, n_nodes, W1, A1, W2, A2, W3, A3):
    from concourse.bass_utils import run_bass_kernel_spmd

    n_nodes = int(n_nodes)
    assert n_nodes == N_NODES
    meta, per_core, nc = _prepare(np.asarray(src), np.asarray(dst), n_nodes)
    npc = meta["npc"]

    w = [build_waug(W1, A1), build_waug(W2, A2), build_waug(W3, A3)]
    h = np.asarray(h, dtype=np.float32)

    in_maps = []
    for c in range(N_CORES):
        in_maps.append(
            {
                "h": np.ascontiguousarray(h[c * npc : (c + 1) * npc]),
                "w0": w[0],
                "w1": w[1],
                "w2": w[2],
                "idxlo": per_core["idxlo"][c],
                "idxhi": per_core["idxhi"][c],
                "seg": per_core["seg"][c],
                "pt0t": per_core["pt0t"][c],
                **const_inputs(),
            }
        )
    res = run_bass_kernel_spmd(nc, in_maps, core_ids=list(range(N_CORES)))
    out = np.concatenate([res.results[c]["out"] for c in range(N_CORES)], axis=0)
    return out[:n_nodes].astype(np.float32)

